# revision 17
# baseline (speedup 1.0000x reference)
"""GraphConv (dgl norm='both') distributed Bass kernel for 8 trn2 NeuronCores.

out = relu( D_in^{-1/2} A D_out^{-1/2} (h W) + b )

Sharding: nodes are range-partitioned across the 8 cores (12500 each, padded
to 12544 = 98*128). Each core:
  phase 1: counts out-degrees of its own nodes from the src-bucketed edge
           list (selection-matrix matmuls against a ones vector),
  phase 2: projects x = (h_shard * norm_src) @ W and appends a ones column,
  phase 3: AllGather of x -> x_full (all 8 shards, in DRAM),
  phase 4: for its dst-bucketed edges: indirect-DMA row gathers of x_full,
           one-hot selection matmuls accumulating [32-node-block, 65] tiles
           in PSUM (the ones column yields the in-degree for free),
  phase 5: per block: norm = rsqrt(max(deg,1)), out = relu(agg * norm) (+b).

Edges are bucketed on the host by (owning core, 32-node cell) with a shared
per-cell chunk-count profile (max across cores) so that all 8 cores run one
identical SPMD program; pad slots use gather row 0 and compare value -1
(whose selection column is all-zero, so they contribute nothing).
"""

import sys

if "/opt/trn_rl_repo" not in sys.path:
    sys.path.insert(0, "/opt/trn_rl_repo")

import numpy as np
from contextlib import ExitStack

import concourse.bass as bass
import concourse.bacc as bacc
import concourse.mybir as mybir
import concourse.tile as tile
from concourse import bass_utils

P = 128
NCORES = 8
N_NODES = 100000
IN_DIM = 256
OUT_DIM = 64
D = OUT_DIM + 1          # features + ones column
NLOC = N_NODES // NCORES  # 12500
GROUPS = 98               # 128-node groups per core
NPAD = GROUPS * P         # 12544
CELL = 32                 # pre-pass cell width
NCELLS = NPAD // CELL     # 392
MCELL = 64                # main-pass cell width
NMCELLS = NPAD // MCELL   # 196
NBANK = 4                 # int16 index banks over x_full rows
NFULL = NCORES * NPAD
BROWS = NFULL // NBANK    # 25088 rows per bank
SBATCH = 32               # chunks per batched S-build
GB = 4096                 # indices per dma_gather instruction (32 chunks)

F32 = mybir.dt.float32
BF16 = mybir.dt.bfloat16
I32 = mybir.dt.int32




def dma_gather_raw(gp, out_ap, in_ap, idxs_ap, num_idxs, elem_size, elem_step):
    """nc.gpsimd.dma_gather without the elem_size%256 restriction (non-transpose,
    DRAM source, all indices valid). elem_step*dtype_size must be a 256B multiple."""
    from concourse._compat import round_up_to_multiple
    from concourse.ap_utils import ap_is_contiguous
    import concourse.mybir as mb

    assert idxs_ap.dtype == mb.dt.int16
    assert in_ap.dtype == out_ap.dtype
    assert in_ap.space == bass.MemorySpace.DRAM
    assert idxs_ap.space == bass.MemorySpace.SBUF
    assert out_ap.space == bass.MemorySpace.SBUF
    assert ap_is_contiguous(out_ap.ap[-1:]) and ap_is_contiguous(idxs_ap.ap[1:])
    assert in_ap.ap[0][0] == elem_step and in_ap.ap[-1][1] == elem_size
    assert out_ap.ap[0][1] * out_ap.ap[1][1] == round_up_to_multiple(num_idxs, 128)
    stride_bytes = elem_step * mb.dt.size(in_ap.dtype)
    stride_bytes_256 = stride_bytes // 256
    assert stride_bytes % 256 == 0 and stride_bytes_256 < 256
    _in_ap = gp.lower_ap_dma(in_ap, for_custom_bir_dma=True)
    _idxs_ap = gp.lower_ap(idxs_ap)
    _out_ap = gp.lower_ap(out_ap)
    return gp.add_instruction(
        mb.InstDMAGatherAnt(
            name=gp.bass.get_next_instruction_name(),
            ins=[*_in_ap, _idxs_ap, gp.lower_val_access(gp.to_reg(num_idxs))],
            outs=[_out_ap],
            transpose=False,
            num_idxs=num_idxs,
            elem_size=elem_size,
            stride_bytes_256=stride_bytes_256,
            gen_mode=0,
            single_packet=False,
            queue_num=0,
            sbuf_tokens_per_rank=0,
            sbuf_free_dim_per_rank=0,
            sbuf_free_dim_pad_per_rank=0,
            sbuf_byte_offset=0,
        )
    )

def _bucket(values_cmp, cells, gidx, ncells):
    """Per-core bucketing: returns (counts, order) with edges sorted by cell."""
    order = np.argsort(cells, kind="stable")
    counts = np.bincount(cells, minlength=ncells)
    return counts, order


def _fill(buf_cmp, buf_gidx, cells_sorted, cmp_sorted, gidx_sorted, counts, off):
    starts = np.concatenate([[0], np.cumsum(counts)[:-1]])
    rank = np.arange(cells_sorted.shape[0]) - starts[cells_sorted]
    pos = off[cells_sorted] + rank
    buf_cmp[pos] = cmp_sorted
    if buf_gidx is not None:
        buf_gidx[pos] = gidx_sorted


def prepare_inputs(h, src, dst, W, b):
    """Host-side sharding / marshalling (layout only, no graph math)."""
    src = np.asarray(src).astype(np.int64)
    dst = np.asarray(dst).astype(np.int64)
    h = np.asarray(h, dtype=np.float32)
    W = np.asarray(W, dtype=np.float32)
    b = np.asarray(b, dtype=np.float32)

    owner_s = src // NLOC
    s_loc = (src - owner_s * NLOC).astype(np.int64)
    grow = (owner_s * NPAD + s_loc).astype(np.int32)  # row in padded x_full
    owner_d = dst // NLOC
    d_loc = (dst - owner_d * NLOC).astype(np.int64)

    # ---- main pass: bucket by (dst owner, dst 64-cell, src bank) ----
    # per-bank edge streams; shared (cell,bank) chunk profile across cores.
    NKEY = NMCELLS * NBANK
    m_counts = np.zeros((NCORES, NKEY), np.int64)
    m_data = []
    for k in range(NCORES):
        m = owner_d == k
        dl = d_loc[m]
        cells = (dl // MCELL).astype(np.int64)
        cmpv = (dl - cells * MCELL).astype(np.float32)
        gi = grow[m]
        bank = gi // BROWS
        key = bank * NMCELLS + cells  # bank-major so each bank is contiguous
        order = np.argsort(key, kind="stable")
        m_counts[k] = np.bincount(key, minlength=NKEY)
        m_data.append((key[order], cmpv[order], (gi - bank * BROWS)[order]))
    nch = np.ceil(m_counts / P).max(axis=0).astype(np.int64)  # [NKEY] shared
    m_off = np.concatenate([[0], np.cumsum(nch)]) * P
    tc_main = int(m_off[-1]) // P          # total chunks over all banks
    tcb = nch.reshape(NBANK, NMCELLS).sum(axis=1).astype(np.int64)  # chunks/bank
    bko = np.concatenate([[0], np.cumsum(tcb)])  # bank chunk offsets

    main_gidx = np.zeros((NCORES, tc_main * P), np.int16)
    main_cmp = np.full((NCORES, tc_main * P), -1.0, np.float32)
    for k in range(NCORES):
        key_s, cmp_s, gi_s = m_data[k]
        _fill(main_cmp[k], main_gidx[k], key_s, cmp_s, gi_s, m_counts[k], m_off[:-1])

    # ---- degree pre-pass: bucket src-locals by (src owner, src 32-cell) ----
    p_counts = np.zeros((NCORES, NCELLS), np.int64)
    p_data = []
    for k in range(NCORES):
        m = owner_s == k
        sl = s_loc[m]
        cells = (sl // CELL).astype(np.int64)
        cmpv = (sl - cells * CELL).astype(np.float32)
        order = np.argsort(cells, kind="stable")
        p_counts[k] = np.bincount(cells, minlength=NCELLS)
        p_data.append((cells[order], cmpv[order]))
    mch = np.ceil(p_counts / P).max(axis=0).astype(np.int64)
    p_off = np.concatenate([[0], np.cumsum(mch)]) * P
    tc_pre = int(p_off[-1]) // P

    pre_cmp = np.full((NCORES, tc_pre * P), -1.0, np.float32)
    for k in range(NCORES):
        cells_s, cmp_s = p_data[k]
        _fill(pre_cmp[k], None, cells_s, cmp_s, None, p_counts[k], p_off[:-1])

    # ---- per-core tensors ----
    import ml_dtypes
    hT = np.zeros((NCORES, IN_DIM, NPAD), ml_dtypes.bfloat16)
    for k in range(NCORES):
        hT[k, :, :NLOC] = h[k * NLOC : (k + 1) * NLOC].T.astype(ml_dtypes.bfloat16)
    iota = np.concatenate([
        np.tile(np.arange(CELL, dtype=np.float32), SBATCH),
        np.tile(np.arange(MCELL, dtype=np.float32), SBATCH),
    ])
    iota_rep = np.broadcast_to(iota, (P, SBATCH * (CELL + MCELL))).copy()
    b_rep = np.broadcast_to(b, (P, OUT_DIM)).copy()

    in_maps = []
    for k in range(NCORES):
        in_maps.append(
            {
                "hT_in": np.ascontiguousarray(hT[k]),
                "W_in": W.astype(ml_dtypes.bfloat16),
                "brep_in": b_rep,
                "iota_in": iota_rep,
                "mgidx_in": np.ascontiguousarray(
                    np.tile(
                        main_gidx[k].reshape(tc_main * P // 16, 16).T, (8, 1)
                    )
                ),
                "mcmp_in": np.ascontiguousarray(main_cmp[k].reshape(tc_main, P).T),
                "pcmp_in": np.ascontiguousarray(pre_cmp[k].reshape(tc_pre, P).T),
            }
        )
    return in_maps, (nch, tcb, bko), mch, tc_main, tc_pre, bool(np.any(b != 0.0))


def build_program(nch_tup, mch, tc_main, tc_pre, has_bias,
                  num_devices=NCORES, phases=(1, 2, 3, 4), compile=True):
    nch, tcb, bko = nch_tup
    """phases: subset of {1: degree pre-pass, 2: x build, 3: allgather,
    4: main gather/scatter + epilogue}. Single-core timing variants replace
    the collective with local DMA copies."""
    nc = bacc.Bacc(
        "TRN2", target_bir_lowering=False, debug=False, num_devices=num_devices
    )

    hT_in = nc.dram_tensor("hT_in", [IN_DIM, NPAD], BF16, kind="ExternalInput")
    W_in = nc.dram_tensor("W_in", [IN_DIM, OUT_DIM], BF16, kind="ExternalInput")
    brep_in = nc.dram_tensor("brep_in", [P, OUT_DIM], F32, kind="ExternalInput")
    iota_in = nc.dram_tensor("iota_in", [P, SBATCH * (CELL + MCELL)], F32, kind="ExternalInput")
    mgidx_in = nc.dram_tensor("mgidx_in", [P, tc_main * 8], mybir.dt.int16, kind="ExternalInput")
    mcmp_in = nc.dram_tensor("mcmp_in", [P, tc_main], F32, kind="ExternalInput")
    pcmp_in = nc.dram_tensor("pcmp_in", [P, tc_pre], F32, kind="ExternalInput")
    out_dram = nc.dram_tensor("out", [NPAD, OUT_DIM], F32, kind="ExternalOutput")

    x_loc = nc.dram_tensor("x_loc", [NPAD, D], BF16)
    x_full = nc.dram_tensor("x_full", [NFULL, D], BF16, addr_space="Shared")
    x_full128 = nc.dram_tensor("x_full128", [NFULL, P], BF16)

    with ExitStack() as ctx:
        tc = ctx.enter_context(tile.TileContext(nc))
        const = ctx.enter_context(tc.tile_pool(name="const", bufs=1))

        # persistent tiles
        iota_t = const.tile([P, SBATCH * (CELL + MCELL)], F32, tag="iota")
        W0 = const.tile([P, OUT_DIM], BF16, tag="W0")
        W1 = const.tile([P, OUT_DIM], BF16, tag="W1")
        ones_t = const.tile([P, 1], F32, tag="ones")
        normsrc = const.tile([P, GROUPS], F32, tag="normsrc")
        pcmp_t = const.tile([P, tc_pre], F32, tag="pcmp")
        mcmp_t = const.tile([P, tc_main], F32, tag="mcmp")
        mgidx_t = const.tile([P, tc_main * 8], mybir.dt.int16, tag="mgidx")
        brep_t = const.tile([P, OUT_DIM], F32, tag="brep")

        nc.sync.dma_start(out=iota_t[:], in_=iota_in[:, :])
        nc.sync.dma_start(out=W0[:], in_=W_in[0:P, :])
        nc.sync.dma_start(out=W1[:], in_=W_in[P : 2 * P, :])
        nc.sync.dma_start(out=pcmp_t[:], in_=pcmp_in[:, :])
        nc.sync.dma_start(out=mcmp_t[:], in_=mcmp_in[:, :])
        nc.sync.dma_start(out=mgidx_t[:], in_=mgidx_in[:, :])
        nc.sync.dma_start(out=brep_t[:], in_=brep_in[:, :])
        nc.vector.memset(ones_t[:], 1.0)

        # ---------------- phase 1: out-degree pre-pass ----------------
        if 1 in phases:
          with (
            tc.tile_pool(name="pre_sb", bufs=4) as pre_sb,
            tc.tile_pool(name="pre_ps", bufs=4, space="PSUM") as pre_ps,
          ):
            j = 0
            Sw = None
            for g in range(GROUPS):
                deg4 = pre_sb.tile([P, 1], F32, tag="deg4")
                for sub in range(4):
                    cell = g * 4 + sub
                    nchunks = int(mch[cell])
                    dps = pre_ps.tile([CELL, 1], F32, space="PSUM", tag="dps")
                    if nchunks == 0:
                        nc.vector.memset(dps[:], 0.0)
                    for c in range(nchunks):
                        if j % SBATCH == 0:
                            w = min(SBATCH, tc_pre - j)
                            Sw = pre_sb.tile([P, SBATCH * CELL], F32, tag="Spre")
                            nc.vector.tensor_tensor(
                                out=Sw[:, : w * CELL],
                                in0=pcmp_t[:, j : j + w].to_broadcast([P, w, CELL]),
                                in1=iota_t[:, : w * CELL],
                                op=mybir.AluOpType.is_equal,
                            )
                        jj = j % SBATCH
                        nc.tensor.matmul(
                            out=dps[:],
                            lhsT=Sw[:, jj * CELL : (jj + 1) * CELL],
                            rhs=ones_t[:],
                            start=(c == 0),
                            stop=(c == nchunks - 1),
                        )
                        j += 1
                    nc.vector.tensor_scalar_max(
                        deg4[CELL * sub : CELL * (sub + 1), :], dps[:], 1.0
                    )
                rcp = pre_sb.tile([P, 1], F32, tag="rcp")
                nc.vector.reciprocal(rcp[:], deg4[:])
                nc.scalar.sqrt(normsrc[:, g : g + 1], rcp[:])
        else:
            nc.vector.memset(normsrc[:], 1.0)

        # ---------------- phase 2: x = (h * norm_src) @ W, ones col ----------------
        if 2 in phases:
          with (
            tc.tile_pool(name="xb_sb", bufs=4) as xb_sb,
            tc.tile_pool(name="xb_ps", bufs=6, space="PSUM") as xb_ps,
          ):
            QUAD = 7  # groups per batched DMA (98 = 14 * 7)
            for q in range(GROUPS // QUAD):
                g0 = q * QUAD
                hta = xb_sb.tile([P, QUAD * P], BF16, tag="hta")
                htb = xb_sb.tile([P, QUAD * P], BF16, tag="htb")
                nc.sync.dma_start(
                    out=hta[:], in_=hT_in[0:P, g0 * P : (g0 + QUAD) * P]
                )
                nc.scalar.dma_start(
                    out=htb[:], in_=hT_in[P : 2 * P, g0 * P : (g0 + QUAD) * P]
                )
                xsb = xb_sb.tile([P, QUAD * D], BF16, tag="xsb")
                for s in range(QUAD):
                    g = g0 + s
                    xps = xb_ps.tile([P, OUT_DIM], F32, space="PSUM", tag="xps")
                    nc.tensor.matmul(
                        out=xps[:], lhsT=hta[:, s * P : (s + 1) * P], rhs=W0[:],
                        start=True, stop=False,
                    )
                    nc.tensor.matmul(
                        out=xps[:], lhsT=htb[:, s * P : (s + 1) * P], rhs=W1[:],
                        start=False, stop=True,
                    )
                    nc.vector.tensor_scalar(
                        out=xsb[:, s * D : s * D + OUT_DIM],
                        in0=xps[:],
                        scalar1=normsrc[:, g : g + 1],
                        scalar2=None,
                        op0=mybir.AluOpType.mult,
                    )
                    nc.vector.memset(xsb[:, s * D + OUT_DIM : (s + 1) * D], 1.0)
                nc.sync.dma_start(
                    out=x_loc.ap()[g0 * P : (g0 + QUAD) * P, :].rearrange(
                        "(a p) d -> p a d", p=P
                    ),
                    in_=xsb[:].rearrange("p (a d) -> p a d", d=D),
                )

        # ---------------- phase 3: AllGather ----------------
        if 3 in phases:
            if num_devices == NCORES:
                nc.gpsimd.collective_compute(
                    "AllGather",
                    mybir.AluOpType.bypass,
                    replica_groups=[list(range(NCORES))],
                    ins=[x_loc.ap().opt()],
                    outs=[x_full.ap().opt()],
                )
            else:
                for k in range(NCORES):
                    nc.sync.dma_start(
                        out=x_full.ap()[k * NPAD : (k + 1) * NPAD, :],
                        in_=x_loc.ap()[:, :],
                    )
        if 4 in phases:
            half = NFULL // 2
            nc.sync.dma_start(
                out=x_full128.ap()[:half, :65], in_=x_full.ap()[:half, :]
            )
            nc.scalar.dma_start(
                out=x_full128.ap()[half:, :65], in_=x_full.ap()[half:, :]
            )

        # ---------------- phase 4+5: gather, scatter matmuls, epilogue ----------------
        if 4 in phases:
          with (
            tc.tile_pool(name="mn_sb", bufs=4) as mn_sb,
            tc.tile_pool(name="mn_msg", bufs=3) as mn_msg,
            tc.tile_pool(name="mn_ps", bufs=6, space="PSUM") as mn_ps,
          ):
            prof = nch.reshape(NBANK, NMCELLS)
            cnt = [0, 0, 0, 0]      # consumed chunks per bank
            Sw = [None] * NBANK
            ww = [None] * NBANK
            OB = 7
            ost = None
            for c in range(NMCELLS):
                g, sub = c // 2, c % 2
                if c % (2 * OB) == 0:
                    ost = mn_sb.tile([P, OB * OUT_DIM], F32, tag="ost")
                if sub == 0:
                    deg2 = mn_sb.tile([P, 1], F32, tag="deg2")
                    accs = []
                so = ((c // 2) % OB) * OUT_DIM
                acc = mn_ps.tile([MCELL, D], F32, space="PSUM", tag="acc")
                accs.append(acc)
                total = int(prof[:, c].sum())
                if total == 0:
                    nc.vector.memset(acc[:], 0.0)
                done = 0
                for b in range(NBANK):
                    for _ in range(int(prof[b, c])):
                        cb = cnt[b]
                        if cb % SBATCH == 0:
                            w = min(SBATCH, int(tcb[b]) - cb)
                            base = (int(bko[b]) + cb)
                            Sw[b] = mn_sb.tile(
                                [P, SBATCH * MCELL], BF16, tag=f"S{b}",
                                name=f"S{b}",
                            )
                            nc.vector.tensor_tensor(
                                out=Sw[b][:, : w * MCELL],
                                in0=mcmp_t[:, base : base + w].to_broadcast(
                                    [P, w, MCELL]
                                ),
                                in1=iota_t[
                                    :,
                                    SBATCH * CELL : SBATCH * CELL + w * MCELL,
                                ],
                                op=mybir.AluOpType.is_equal,
                            )
                            ww[b] = mn_msg.tile(
                                [P, SBATCH * D], BF16, tag=f"w{b}",
                                name=f"w{b}",
                            )
                            dma_gather_raw(
                                nc.gpsimd,
                                out_ap=ww[b][:, : w * D].rearrange(
                                    "p (k d) -> p k d", d=D
                                ),
                                in_ap=x_full128.ap()[
                                    b * BROWS : (b + 1) * BROWS, :D
                                ],
                                idxs_ap=mgidx_t[
                                    :, base * 8 : (base + w) * 8
                                ],
                                num_idxs=w * P,
                                elem_size=D,
                                elem_step=P,
                            )
                        jj = cb % SBATCH
                        nc.tensor.matmul(
                            out=acc[:],
                            lhsT=Sw[b][:, jj * MCELL : (jj + 1) * MCELL],
                            rhs=ww[b][:, jj * D : (jj + 1) * D],
                            start=(done == 0),
                            stop=(done == total - 1),
                        )
                        done += 1
                        cnt[b] = cb + 1
                nc.vector.tensor_scalar_max(
                    deg2[MCELL * sub : MCELL * (sub + 1), :],
                    acc[:, OUT_DIM:D],
                    1.0,
                )
                if sub == 1:
                    rcp = mn_sb.tile([P, 1], F32, tag="rcpm")
                    norm2 = mn_sb.tile([P, 1], F32, tag="norm2")
                    nc.vector.reciprocal(rcp[:], deg2[:])
                    nc.scalar.sqrt(norm2[:], rcp[:])
                    for s2 in range(2):
                        sl = slice(MCELL * s2, MCELL * (s2 + 1))
                        osl = ost[sl, so : so + OUT_DIM]
                        if has_bias:
                            nc.vector.tensor_scalar(
                                out=osl,
                                in0=accs[s2][:, :OUT_DIM],
                                scalar1=norm2[sl, :],
                                scalar2=None,
                                op0=mybir.AluOpType.mult,
                            )
                            nc.vector.tensor_tensor(
                                out=osl, in0=osl, in1=brep_t[sl, :],
                                op=mybir.AluOpType.add,
                            )
                            nc.scalar.activation(
                                osl, osl, mybir.ActivationFunctionType.Relu
                            )
                        else:
                            nc.scalar.activation(
                                osl,
                                accs[s2][:, :OUT_DIM],
                                mybir.ActivationFunctionType.Relu,
                                scale=norm2[sl, :],
                            )
                    if g % OB == OB - 1:
                        g0 = g - (OB - 1)
                        nc.sync.dma_start(
                            out=out_dram.ap()[
                                g0 * P : (g0 + OB) * P, :
                            ].rearrange("(a p) d -> p a d", p=P),
                            in_=ost[:].rearrange("p (a d) -> p a d", d=OUT_DIM),
                        )

    if compile:
        nc.compile()
    return nc


def kernel(h, src, dst, W, b):
    in_maps, nch, mch, tc_main, tc_pre, has_bias = prepare_inputs(h, src, dst, W, b)
    nc = build_program(nch, mch, tc_main, tc_pre, has_bias)
    res = bass_utils.run_bass_kernel_spmd(
        nc, in_maps, core_ids=list(range(NCORES))
    )
    out = np.concatenate(
        [res.results[k]["out"][:NLOC] for k in range(NCORES)], axis=0
    )
    return out.astype(np.float32)


# revision 19
# speedup vs baseline: 28.0629x; 28.0629x over previous
"""GraphConv (dgl norm='both') distributed Bass kernel for 8 trn2 NeuronCores.

out = relu( D_in^{-1/2} A D_out^{-1/2} (h W) + b )

Sharding: nodes are range-partitioned across the 8 cores (12500 each, padded
to 12544 = 98*128). Each core:
  phase 1: counts out-degrees of its own nodes from the src-bucketed edge
           list (selection-matrix matmuls against a ones vector),
  phase 2: projects x = (h_shard * norm_src) @ W and appends a ones column,
  phase 3: AllGather of x -> x_full (all 8 shards, in DRAM),
  phase 4: for its dst-bucketed edges: indirect-DMA row gathers of x_full,
           one-hot selection matmuls accumulating [32-node-block, 65] tiles
           in PSUM (the ones column yields the in-degree for free),
  phase 5: per block: norm = rsqrt(max(deg,1)), out = relu(agg * norm) (+b).

Edges are bucketed on the host by (owning core, 32-node cell) with a shared
per-cell chunk-count profile (max across cores) so that all 8 cores run one
identical SPMD program; pad slots use gather row 0 and compare value -1
(whose selection column is all-zero, so they contribute nothing).
"""

import sys

if "/opt/trn_rl_repo" not in sys.path:
    sys.path.insert(0, "/opt/trn_rl_repo")

import numpy as np
from contextlib import ExitStack

import concourse.bass as bass
import concourse.bacc as bacc
import concourse.mybir as mybir
import concourse.tile as tile
from concourse import bass_utils

P = 128
NCORES = 8
N_NODES = 100000
IN_DIM = 256
OUT_DIM = 64
D = OUT_DIM + 1          # features + ones column
NLOC = N_NODES // NCORES  # 12500
GROUPS = 98               # 128-node groups per core
NPAD = GROUPS * P         # 12544
CELL = 32                 # pre-pass cell width
NCELLS = NPAD // CELL     # 392
MCELL = 64                # main-pass cell width
NMCELLS = NPAD // MCELL   # 196
NBANK = 4                 # int16 index banks over x_full rows
NFULL = NCORES * NPAD
BROWS = NFULL // NBANK    # 25088 rows per bank
SBATCH = 32               # chunks per batched S-build
GB = 4096                 # indices per dma_gather instruction (32 chunks)

F32 = mybir.dt.float32
BF16 = mybir.dt.bfloat16
I32 = mybir.dt.int32




def dma_gather_raw(gp, out_ap, in_ap, idxs_ap, num_idxs, elem_size, elem_step):
    """nc.gpsimd.dma_gather without the elem_size%256 restriction (non-transpose,
    DRAM source, all indices valid). elem_step*dtype_size must be a 256B multiple."""
    from concourse._compat import round_up_to_multiple
    from concourse.ap_utils import ap_is_contiguous
    import concourse.mybir as mb

    assert idxs_ap.dtype == mb.dt.int16
    assert in_ap.dtype == out_ap.dtype
    assert in_ap.space == bass.MemorySpace.DRAM
    assert idxs_ap.space == bass.MemorySpace.SBUF
    assert out_ap.space == bass.MemorySpace.SBUF
    assert ap_is_contiguous(out_ap.ap[-1:]) and ap_is_contiguous(idxs_ap.ap[1:])
    assert in_ap.ap[0][0] == elem_step and in_ap.ap[-1][1] == elem_size
    assert out_ap.ap[0][1] * out_ap.ap[1][1] == round_up_to_multiple(num_idxs, 128)
    stride_bytes = elem_step * mb.dt.size(in_ap.dtype)
    stride_bytes_256 = stride_bytes // 256
    assert stride_bytes % 256 == 0 and stride_bytes_256 < 256
    _in_ap = gp.lower_ap_dma(in_ap, for_custom_bir_dma=True)
    _idxs_ap = gp.lower_ap(idxs_ap)
    _out_ap = gp.lower_ap(out_ap)
    return gp.add_instruction(
        mb.InstDMAGatherAnt(
            name=gp.bass.get_next_instruction_name(),
            ins=[*_in_ap, _idxs_ap, gp.lower_val_access(gp.to_reg(num_idxs))],
            outs=[_out_ap],
            transpose=False,
            num_idxs=num_idxs,
            elem_size=elem_size,
            stride_bytes_256=stride_bytes_256,
            gen_mode=0,
            single_packet=False,
            queue_num=0,
            sbuf_tokens_per_rank=0,
            sbuf_free_dim_per_rank=0,
            sbuf_free_dim_pad_per_rank=0,
            sbuf_byte_offset=0,
        )
    )

def _bucket(values_cmp, cells, gidx, ncells):
    """Per-core bucketing: returns (counts, order) with edges sorted by cell."""
    order = np.argsort(cells, kind="stable")
    counts = np.bincount(cells, minlength=ncells)
    return counts, order


def _fill(buf_cmp, buf_gidx, cells_sorted, cmp_sorted, gidx_sorted, counts, off):
    starts = np.concatenate([[0], np.cumsum(counts)[:-1]])
    rank = np.arange(cells_sorted.shape[0]) - starts[cells_sorted]
    pos = off[cells_sorted] + rank
    buf_cmp[pos] = cmp_sorted
    if buf_gidx is not None:
        buf_gidx[pos] = gidx_sorted


def prepare_inputs(h, src, dst, W, b):
    """Host-side sharding / marshalling (layout only, no graph math)."""
    src = np.asarray(src).astype(np.int64)
    dst = np.asarray(dst).astype(np.int64)
    h = np.asarray(h, dtype=np.float32)
    W = np.asarray(W, dtype=np.float32)
    b = np.asarray(b, dtype=np.float32)

    owner_s = src // NLOC
    s_loc = (src - owner_s * NLOC).astype(np.int64)
    grow = (owner_s * NPAD + s_loc).astype(np.int32)  # row in padded x_full
    owner_d = dst // NLOC
    d_loc = (dst - owner_d * NLOC).astype(np.int64)

    # ---- main pass: bucket by (dst owner, dst 64-cell, src bank) ----
    # per-bank edge streams; shared (cell,bank) chunk profile across cores.
    NKEY = NMCELLS * NBANK
    m_counts = np.zeros((NCORES, NKEY), np.int64)
    m_data = []
    for k in range(NCORES):
        m = owner_d == k
        dl = d_loc[m]
        cells = (dl // MCELL).astype(np.int64)
        cmpv = (dl - cells * MCELL).astype(np.float32)
        gi = grow[m]
        bank = gi // BROWS
        key = bank * NMCELLS + cells  # bank-major so each bank is contiguous
        order = np.argsort(key, kind="stable")
        m_counts[k] = np.bincount(key, minlength=NKEY)
        m_data.append((key[order], cmpv[order], (gi - bank * BROWS)[order]))
    nch = np.ceil(m_counts / P).max(axis=0).astype(np.int64)  # [NKEY] shared
    m_off = np.concatenate([[0], np.cumsum(nch)]) * P
    tc_main = int(m_off[-1]) // P          # total chunks over all banks
    tcb = nch.reshape(NBANK, NMCELLS).sum(axis=1).astype(np.int64)  # chunks/bank
    bko = np.concatenate([[0], np.cumsum(tcb)])  # bank chunk offsets

    main_gidx = np.zeros((NCORES, tc_main * P), np.int16)
    main_cmp = np.full((NCORES, tc_main * P), -1.0, np.float32)
    for k in range(NCORES):
        key_s, cmp_s, gi_s = m_data[k]
        _fill(main_cmp[k], main_gidx[k], key_s, cmp_s, gi_s, m_counts[k], m_off[:-1])

    # ---- degree pre-pass: bucket src-locals by (src owner, src 32-cell) ----
    p_counts = np.zeros((NCORES, NCELLS), np.int64)
    p_data = []
    for k in range(NCORES):
        m = owner_s == k
        sl = s_loc[m]
        cells = (sl // CELL).astype(np.int64)
        cmpv = (sl - cells * CELL).astype(np.float32)
        order = np.argsort(cells, kind="stable")
        p_counts[k] = np.bincount(cells, minlength=NCELLS)
        p_data.append((cells[order], cmpv[order]))
    mch = np.ceil(p_counts / P).max(axis=0).astype(np.int64)
    p_off = np.concatenate([[0], np.cumsum(mch)]) * P
    tc_pre = int(p_off[-1]) // P

    pre_cmp = np.full((NCORES, tc_pre * P), -1.0, np.float32)
    for k in range(NCORES):
        cells_s, cmp_s = p_data[k]
        _fill(pre_cmp[k], None, cells_s, cmp_s, None, p_counts[k], p_off[:-1])

    # ---- per-core tensors ----
    import ml_dtypes
    hT = np.zeros((NCORES, IN_DIM, NPAD), ml_dtypes.bfloat16)
    for k in range(NCORES):
        hT[k, :, :NLOC] = h[k * NLOC : (k + 1) * NLOC].T.astype(ml_dtypes.bfloat16)
    iota = np.concatenate([
        np.tile(np.arange(CELL, dtype=np.float32), SBATCH),
        np.tile(np.arange(MCELL, dtype=np.float32), SBATCH),
    ])
    iota_rep = np.broadcast_to(iota, (P, SBATCH * (CELL + MCELL))).copy()
    b_rep = np.broadcast_to(b, (P, OUT_DIM)).copy()

    in_maps = []
    for k in range(NCORES):
        in_maps.append(
            {
                "hT_in": np.ascontiguousarray(hT[k]),
                "W_in": W.astype(ml_dtypes.bfloat16),
                "brep_in": b_rep,
                "iota_in": iota_rep,
                "mgidx_in": np.ascontiguousarray(
                    np.tile(
                        main_gidx[k].reshape(tc_main * P // 16, 16).T, (8, 1)
                    )
                ),
                "mcmp_in": np.ascontiguousarray(main_cmp[k].reshape(tc_main, P).T),
                "pcmp_in": np.ascontiguousarray(pre_cmp[k].reshape(tc_pre, P).T),
            }
        )
    return in_maps, (nch, tcb, bko), mch, tc_main, tc_pre, bool(np.any(b != 0.0))


def build_program(nch_tup, mch, tc_main, tc_pre, has_bias,
                  num_devices=NCORES, phases=(1, 2, 3, 4), compile=True,
                  repeat=1, ag_only=0):
    nch, tcb, bko = nch_tup
    """phases: subset of {1: degree pre-pass, 2: x build, 3: allgather,
    4: main gather/scatter + epilogue}. Single-core timing variants replace
    the collective with local DMA copies."""
    nc = bacc.Bacc(
        "TRN2", target_bir_lowering=False, debug=False, num_devices=num_devices
    )

    hT_in = nc.dram_tensor("hT_in", [IN_DIM, NPAD], BF16, kind="ExternalInput")
    W_in = nc.dram_tensor("W_in", [IN_DIM, OUT_DIM], BF16, kind="ExternalInput")
    brep_in = nc.dram_tensor("brep_in", [P, OUT_DIM], F32, kind="ExternalInput")
    iota_in = nc.dram_tensor("iota_in", [P, SBATCH * (CELL + MCELL)], F32, kind="ExternalInput")
    mgidx_in = nc.dram_tensor("mgidx_in", [P, tc_main * 8], mybir.dt.int16, kind="ExternalInput")
    mcmp_in = nc.dram_tensor("mcmp_in", [P, tc_main], F32, kind="ExternalInput")
    pcmp_in = nc.dram_tensor("pcmp_in", [P, tc_pre], F32, kind="ExternalInput")
    out_dram = nc.dram_tensor("out", [NPAD, OUT_DIM], F32, kind="ExternalOutput")

    x_loc = nc.dram_tensor("x_loc", [NPAD, D], BF16)
    x_full = nc.dram_tensor("x_full", [NFULL, D], BF16, addr_space="Shared")
    x_full128 = nc.dram_tensor("x_full128", [NFULL, P], BF16)

    with ExitStack() as ctx:
        tc = ctx.enter_context(tile.TileContext(nc))
        const = ctx.enter_context(tc.tile_pool(name="const", bufs=1))

        # persistent tiles
        iota_t = const.tile([P, SBATCH * (CELL + MCELL)], F32, tag="iota")
        W0 = const.tile([P, OUT_DIM], BF16, tag="W0")
        W1 = const.tile([P, OUT_DIM], BF16, tag="W1")
        ones_t = const.tile([P, 1], F32, tag="ones")
        normsrc = const.tile([P, GROUPS], F32, tag="normsrc")
        pcmp_t = const.tile([P, tc_pre], F32, tag="pcmp")
        mcmp_t = const.tile([P, tc_main], F32, tag="mcmp")
        mgidx_t = const.tile([P, tc_main * 8], mybir.dt.int16, tag="mgidx")
        brep_t = const.tile([P, OUT_DIM], F32, tag="brep")

        nc.sync.dma_start(out=iota_t[:], in_=iota_in[:, :])
        nc.sync.dma_start(out=W0[:], in_=W_in[0:P, :])
        nc.sync.dma_start(out=W1[:], in_=W_in[P : 2 * P, :])
        nc.sync.dma_start(out=pcmp_t[:], in_=pcmp_in[:, :])
        nc.sync.dma_start(out=mcmp_t[:], in_=mcmp_in[:, :])
        nc.sync.dma_start(out=mgidx_t[:], in_=mgidx_in[:, :])
        nc.sync.dma_start(out=brep_t[:], in_=brep_in[:, :])
        nc.vector.memset(ones_t[:], 1.0)

        if ag_only:
            for _ in range(ag_only):
                nc.gpsimd.collective_compute(
                    "AllGather",
                    mybir.AluOpType.bypass,
                    replica_groups=[list(range(NCORES))],
                    ins=[x_loc.ap().opt()],
                    outs=[x_full.ap().opt()],
                )
            dummy = const.tile([P, OUT_DIM], BF16, tag="dummy")
            dummy2 = const.tile([P, OUT_DIM], F32, tag="dummy2")
            nc.sync.dma_start(out=dummy[:], in_=x_full.ap()[:P, :OUT_DIM])
            nc.vector.tensor_copy(out=dummy2[:], in_=dummy[:])
            nc.sync.dma_start(out=out_dram[:P, :], in_=dummy2[:])
            nc.compile()
            return nc

        rep_ctx = tc.For_i(0, repeat, 1) if repeat > 1 else None
        if rep_ctx is not None:
            rep_ctx.__enter__()

        # ---------------- phase 1: out-degree pre-pass ----------------
        if 1 in phases:
          with (
            tc.tile_pool(name="pre_sb", bufs=4) as pre_sb,
            tc.tile_pool(name="pre_ps", bufs=4, space="PSUM") as pre_ps,
          ):
            j = 0
            Sw = None
            for g in range(GROUPS):
                deg4 = pre_sb.tile([P, 1], F32, tag="deg4")
                for sub in range(4):
                    cell = g * 4 + sub
                    nchunks = int(mch[cell])
                    dps = pre_ps.tile([CELL, 1], F32, space="PSUM", tag="dps")
                    if nchunks == 0:
                        nc.vector.memset(dps[:], 0.0)
                    for c in range(nchunks):
                        if j % SBATCH == 0:
                            w = min(SBATCH, tc_pre - j)
                            Sw = pre_sb.tile([P, SBATCH * CELL], F32, tag="Spre")
                            nc.vector.tensor_tensor(
                                out=Sw[:, : w * CELL],
                                in0=pcmp_t[:, j : j + w].to_broadcast([P, w, CELL]),
                                in1=iota_t[:, : w * CELL],
                                op=mybir.AluOpType.is_equal,
                            )
                        jj = j % SBATCH
                        nc.tensor.matmul(
                            out=dps[:],
                            lhsT=Sw[:, jj * CELL : (jj + 1) * CELL],
                            rhs=ones_t[:],
                            start=(c == 0),
                            stop=(c == nchunks - 1),
                        )
                        j += 1
                    nc.vector.tensor_scalar_max(
                        deg4[CELL * sub : CELL * (sub + 1), :], dps[:], 1.0
                    )
                rcp = pre_sb.tile([P, 1], F32, tag="rcp")
                nc.vector.reciprocal(rcp[:], deg4[:])
                nc.scalar.sqrt(normsrc[:, g : g + 1], rcp[:])
        else:
            nc.vector.memset(normsrc[:], 1.0)

        # ---------------- phase 2: x = (h * norm_src) @ W, ones col ----------------
        if 2 in phases:
          with (
            tc.tile_pool(name="xb_sb", bufs=4) as xb_sb,
            tc.tile_pool(name="xb_ps", bufs=6, space="PSUM") as xb_ps,
          ):
            QUAD = 7  # groups per batched DMA (98 = 14 * 7)
            for q in range(GROUPS // QUAD):
                g0 = q * QUAD
                hta = xb_sb.tile([P, QUAD * P], BF16, tag="hta")
                htb = xb_sb.tile([P, QUAD * P], BF16, tag="htb")
                nc.sync.dma_start(
                    out=hta[:], in_=hT_in[0:P, g0 * P : (g0 + QUAD) * P]
                )
                nc.scalar.dma_start(
                    out=htb[:], in_=hT_in[P : 2 * P, g0 * P : (g0 + QUAD) * P]
                )
                xsb = xb_sb.tile([P, QUAD * D], BF16, tag="xsb")
                for s in range(QUAD):
                    g = g0 + s
                    xps = xb_ps.tile([P, OUT_DIM], F32, space="PSUM", tag="xps")
                    nc.tensor.matmul(
                        out=xps[:], lhsT=hta[:, s * P : (s + 1) * P], rhs=W0[:],
                        start=True, stop=False,
                    )
                    nc.tensor.matmul(
                        out=xps[:], lhsT=htb[:, s * P : (s + 1) * P], rhs=W1[:],
                        start=False, stop=True,
                    )
                    nc.vector.tensor_scalar(
                        out=xsb[:, s * D : s * D + OUT_DIM],
                        in0=xps[:],
                        scalar1=normsrc[:, g : g + 1],
                        scalar2=None,
                        op0=mybir.AluOpType.mult,
                    )
                    nc.vector.memset(xsb[:, s * D + OUT_DIM : (s + 1) * D], 1.0)
                nc.sync.dma_start(
                    out=x_loc.ap()[g0 * P : (g0 + QUAD) * P, :].rearrange(
                        "(a p) d -> p a d", p=P
                    ),
                    in_=xsb[:].rearrange("p (a d) -> p a d", d=D),
                )

        # ---------------- phase 3: AllGather ----------------
        if 3 in phases and repeat == 1:
            if num_devices == NCORES:
                nc.gpsimd.collective_compute(
                    "AllGather",
                    mybir.AluOpType.bypass,
                    replica_groups=[list(range(NCORES))],
                    ins=[x_loc.ap().opt()],
                    outs=[x_full.ap().opt()],
                )
            else:
                for k in range(NCORES):
                    nc.sync.dma_start(
                        out=x_full.ap()[k * NPAD : (k + 1) * NPAD, :],
                        in_=x_loc.ap()[:, :],
                    )
        if 4 in phases:
            half = NFULL // 2
            nc.sync.dma_start(
                out=x_full128.ap()[:half, :65], in_=x_full.ap()[:half, :]
            )
            nc.scalar.dma_start(
                out=x_full128.ap()[half:, :65], in_=x_full.ap()[half:, :]
            )

        # ---------------- phase 4+5: gather, scatter matmuls, epilogue ----------------
        if 4 in phases:
          with (
            tc.tile_pool(name="mn_sb", bufs=4) as mn_sb,
            tc.tile_pool(name="mn_msg", bufs=3) as mn_msg,
            tc.tile_pool(name="mn_ps", bufs=6, space="PSUM") as mn_ps,
          ):
            prof = nch.reshape(NBANK, NMCELLS)
            cnt = [0, 0, 0, 0]      # consumed chunks per bank
            Sw = [None] * NBANK
            ww = [None] * NBANK
            OB = 7
            ost = None
            for c in range(NMCELLS):
                g, sub = c // 2, c % 2
                if c % (2 * OB) == 0:
                    ost = mn_sb.tile([P, OB * OUT_DIM], F32, tag="ost")
                if sub == 0:
                    deg2 = mn_sb.tile([P, 1], F32, tag="deg2")
                    accs = []
                so = ((c // 2) % OB) * OUT_DIM
                acc = mn_ps.tile([MCELL, D], F32, space="PSUM", tag="acc")
                accs.append(acc)
                total = int(prof[:, c].sum())
                if total == 0:
                    nc.vector.memset(acc[:], 0.0)
                done = 0
                for b in range(NBANK):
                    for _ in range(int(prof[b, c])):
                        cb = cnt[b]
                        if cb % SBATCH == 0:
                            w = min(SBATCH, int(tcb[b]) - cb)
                            base = (int(bko[b]) + cb)
                            Sw[b] = mn_sb.tile(
                                [P, SBATCH * MCELL], BF16, tag=f"S{b}",
                                name=f"S{b}",
                            )
                            nc.vector.tensor_tensor(
                                out=Sw[b][:, : w * MCELL],
                                in0=mcmp_t[:, base : base + w].to_broadcast(
                                    [P, w, MCELL]
                                ),
                                in1=iota_t[
                                    :,
                                    SBATCH * CELL : SBATCH * CELL + w * MCELL,
                                ],
                                op=mybir.AluOpType.is_equal,
                            )
                            ww[b] = mn_msg.tile(
                                [P, SBATCH * D], BF16, tag=f"w{b}",
                                name=f"w{b}",
                            )
                            dma_gather_raw(
                                nc.gpsimd,
                                out_ap=ww[b][:, : w * D].rearrange(
                                    "p (k d) -> p k d", d=D
                                ),
                                in_ap=x_full128.ap()[
                                    b * BROWS : (b + 1) * BROWS, :D
                                ],
                                idxs_ap=mgidx_t[
                                    :, base * 8 : (base + w) * 8
                                ],
                                num_idxs=w * P,
                                elem_size=D,
                                elem_step=P,
                            )
                        jj = cb % SBATCH
                        nc.tensor.matmul(
                            out=acc[:],
                            lhsT=Sw[b][:, jj * MCELL : (jj + 1) * MCELL],
                            rhs=ww[b][:, jj * D : (jj + 1) * D],
                            start=(done == 0),
                            stop=(done == total - 1),
                        )
                        done += 1
                        cnt[b] = cb + 1
                nc.vector.tensor_scalar_max(
                    deg2[MCELL * sub : MCELL * (sub + 1), :],
                    acc[:, OUT_DIM:D],
                    1.0,
                )
                if sub == 1:
                    rcp = mn_sb.tile([P, 1], F32, tag="rcpm")
                    norm2 = mn_sb.tile([P, 1], F32, tag="norm2")
                    nc.vector.reciprocal(rcp[:], deg2[:])
                    nc.scalar.sqrt(norm2[:], rcp[:])
                    for s2 in range(2):
                        sl = slice(MCELL * s2, MCELL * (s2 + 1))
                        osl = ost[sl, so : so + OUT_DIM]
                        if has_bias:
                            nc.vector.tensor_scalar(
                                out=osl,
                                in0=accs[s2][:, :OUT_DIM],
                                scalar1=norm2[sl, :],
                                scalar2=None,
                                op0=mybir.AluOpType.mult,
                            )
                            nc.vector.tensor_tensor(
                                out=osl, in0=osl, in1=brep_t[sl, :],
                                op=mybir.AluOpType.add,
                            )
                            nc.scalar.activation(
                                osl, osl, mybir.ActivationFunctionType.Relu
                            )
                        else:
                            nc.scalar.activation(
                                osl,
                                accs[s2][:, :OUT_DIM],
                                mybir.ActivationFunctionType.Relu,
                                scale=norm2[sl, :],
                            )
                    if g % OB == OB - 1:
                        g0 = g - (OB - 1)
                        nc.sync.dma_start(
                            out=out_dram.ap()[
                                g0 * P : (g0 + OB) * P, :
                            ].rearrange("(a p) d -> p a d", p=P),
                            in_=ost[:].rearrange("p (a d) -> p a d", d=OUT_DIM),
                        )

        if rep_ctx is not None:
            rep_ctx.__exit__(None, None, None)

    if compile:
        nc.compile()
    return nc


def kernel(h, src, dst, W, b):
    in_maps, nch, mch, tc_main, tc_pre, has_bias = prepare_inputs(h, src, dst, W, b)
    nc = build_program(nch, mch, tc_main, tc_pre, has_bias)
    res = bass_utils.run_bass_kernel_spmd(
        nc, in_maps, core_ids=list(range(NCORES))
    )
    out = np.concatenate(
        [res.results[k]["out"][:NLOC] for k in range(NCORES)], axis=0
    )
    return out.astype(np.float32)
